# revision 3
# baseline (speedup 1.0000x reference)
"""Expert-parallel sparse top-2 MoE on 8 TRN2 NeuronCores.

One expert per core. Routing/top-2/softmax run on host (bit-matching the
reference's jax-on-CPU ops); each expert's routed tokens are pre-gathered,
pre-gated (g*x, valid since softmax weights are positive and relu is
positively homogeneous), transposed to K-major fp16, and shipped to the
expert's core. The core runs the FFN (mm1 [K=1024]->relu->mm2 [K=4096])
over CAP token slots, indirect-scatters output rows into a per-destination
padded send buffer, and one 8-core AllToAll delivers every token's two
expert contributions to its owner core, which gathers both rows and adds.

Weight DMA per core drops 8x vs token-parallel (only its own expert).
"""

import os

import numpy as np

NUM_EXPERTS = 8
D = 1024
F = 4096
B, S = 2, 2048
T = B * S
N_CORES = 8
TPC = T // N_CORES  # tokens owned per core (output shard)

LAST_RESULT = None
_NC_CACHE = {}


def _build_nc(cap, capd):
    import concourse.mybir as mybir
    import concourse.tile as tile
    from concourse import bacc, bass

    dt = mybir.dt
    au = mybir.AluOpType
    nrow = N_CORES * capd
    sc_n = cap // 128  # slot chunks
    nc = bacc.Bacc("TRN2", target_bir_lowering=False, debug=False, num_devices=N_CORES)

    xgt_d = nc.dram_tensor("xgt", [128, 8, cap], dt.float16, kind="ExternalInput").ap()
    w1_d = nc.dram_tensor("w1p", [8, 128, 4, 8, 128], dt.float16, kind="ExternalInput").ap()
    w2_d = nc.dram_tensor("w2p", [8, 128, 4, D], dt.float16, kind="ExternalInput").ap()
    sidx_d = nc.dram_tensor("sidx", [128, sc_n], dt.int32, kind="ExternalInput").ap()
    g1_d = nc.dram_tensor("g1", [128, 4], dt.int32, kind="ExternalInput").ap()
    g2_d = nc.dram_tensor("g2", [128, 4], dt.int32, kind="ExternalInput").ap()
    out_d = nc.dram_tensor("out", [TPC, D], dt.float32, kind="ExternalOutput").ap()

    # mm1 moving-dim blocks covering cap columns
    nblocks = []
    off = 0
    while off < cap:
        nn = min(512, cap - off)
        nblocks.append((off, nn))
        off += nn
    # mm2 slot-chunk passes (3 chunks x 2 halves = 6 PSUM banks; a 3-chunk
    # pass keeps per-ft matmul work above the W2 stream DMA time)
    passes = [list(range(i, min(i + 3, sc_n))) for i in range(0, sc_n, 3)]

    with tile.TileContext(nc) as tc:
        with (
            tc.tile_pool(name="res", bufs=1) as res,
            tc.tile_pool(name="w1pool", bufs=2) as w1pool,
            tc.tile_pool(name="w2pool", bufs=2) as w2pool,
            tc.tile_pool(name="ogpool", bufs=2) as ogpool,
            tc.tile_pool(name="dram", bufs=1, space="DRAM") as drampool,
        ):
            send = drampool.tile([nrow, D], dt.float16, tag="send")
            recv = drampool.tile([nrow, D], dt.float16, tag="recv")

            # Warm up the collectives path concurrently with compute: the
            # first collective after NEFF load pays ~100us of firmware
            # init/sync; a tiny dummy AllToAll absorbs it under mm1.
            dummy_s = drampool.tile([N_CORES, 16], dt.float32, tag="dummy_s")
            dummy_r = drampool.tile([N_CORES, 16], dt.float32, tag="dummy_r")
            nc.gpsimd.collective_compute(
                "AllToAll",
                au.bypass,
                replica_groups=[list(range(N_CORES))],
                ins=[dummy_s.opt()],
                outs=[dummy_r.opt()],
            )

            # Inputs ride separate engine queues so the first W1 chunk (on
            # sync) isn't queued behind the 2.4 MB XgT load: mm1 can start
            # as soon as w1[fc0] + xgt[ko0] land. Per-ko XgT chunks let the
            # fc0 accumulation begin before the full activation load.
            XgT = res.tile([128, 8, cap], dt.float16)
            for ko in range(8):
                nc.scalar.dma_start(XgT[:, ko, :], xgt_d[:, ko, :])
            SIDX = res.tile([128, sc_n], dt.int32)
            nc.gpsimd.dma_start(SIDX[:], sidx_d[:])
            G1 = res.tile([128, 4], dt.int32)
            nc.gpsimd.dma_start(G1[:], g1_d[:])
            G2 = res.tile([128, 4], dt.int32)
            nc.gpsimd.dma_start(G2[:], g2_d[:])

            Hg = res.tile([128, 32, cap], dt.float16)

            # ---- mm1 + relu: Hg[f, slot] = relu(w1.T @ xg) ----
            with tc.tile_pool(name="psum_h", bufs=2, space="PSUM") as psum_h:
                for fc in range(32):
                    if fc % 4 == 0:
                        W1C4 = w1pool.tile([128, 4, 8, 128], dt.float16, tag="w1c")
                        nc.sync.dma_start(W1C4[:], w1_d[fc // 4])
                    phs = [
                        psum_h.tile(
                            [128, nn], dt.float32, tag=f"ph{nb}", name=f"ph{nb}"
                        )
                        for nb, (_, nn) in enumerate(nblocks)
                    ]
                    for ko in range(8):
                        for nb, (n0, nn) in enumerate(nblocks):
                            nc.tensor.matmul(
                                phs[nb][:],
                                W1C4[:, fc % 4, ko, :],
                                XgT[:, ko, n0 : n0 + nn],
                                start=(ko == 0),
                                stop=(ko == 7),
                            )
                    for nb, (n0, nn) in enumerate(nblocks):
                        nc.scalar.activation(
                            Hg[:, fc, n0 : n0 + nn],
                            phs[nb][:],
                            mybir.ActivationFunctionType.Relu,
                        )

            # ---- mm2: out rows per slot chunk; scatter to send ----
            with tc.tile_pool(name="psum_o", bufs=1, space="PSUM") as psum_o:
                for pi, scs in enumerate(passes):
                    pos = {}
                    for sc in scs:
                        for dh in range(2):
                            # rotate across 4 tag groups (8 banks) so a new
                            # pass accumulates into banks the previous pass
                            # isn't still evacuating
                            tg = (3 * pi + (sc - scs[0])) % 4
                            pos[(sc, dh)] = psum_o.tile(
                                [128, 512],
                                dt.float32,
                                tag=f"po{tg}_{dh}",
                                name=f"po{tg}_{dh}",
                            )
                    for ft in range(32):
                        if ft % 4 == 0:
                            W2T4 = w2pool.tile([128, 4, D], dt.float16, tag="w2t")
                            nc.sync.dma_start(W2T4[:], w2_d[ft // 4])
                        for sc in scs:
                            for dh in range(2):
                                nc.tensor.matmul(
                                    pos[(sc, dh)][:],
                                    Hg[:, ft, sc * 128 : (sc + 1) * 128],
                                    W2T4[:, ft % 4, dh * 512 : (dh + 1) * 512],
                                    start=(ft == 0),
                                    stop=(ft == 31),
                                )
                    for sc in scs:
                        OGC = ogpool.tile([128, D], dt.float16, tag="og")
                        # one half per engine so both evacuate in parallel
                        nc.vector.tensor_copy(OGC[:, 0:512], pos[(sc, 0)][:])
                        nc.scalar.copy(OGC[:, 512:1024], pos[(sc, 1)][:])
                        nc.gpsimd.indirect_dma_start(
                            out=send[:],
                            out_offset=bass.IndirectOffsetOnAxis(
                                ap=SIDX[:, sc : sc + 1], axis=0
                            ),
                            in_=OGC[:],
                            in_offset=None,
                            bounds_check=nrow - 1,
                            oob_is_err=False,
                        )

            # ---- AllToAll: deliver rows to token-owner cores ----
            nc.gpsimd.collective_compute(
                "AllToAll",
                au.bypass,
                replica_groups=[list(range(N_CORES))],
                ins=[send.opt()],
                outs=[recv.opt()],
            )

            # ---- combine: out[t] = recv[g1[t]] + recv[g2[t]] ----
            GB1 = res.tile([128, 4, D], dt.float16)
            GB2 = res.tile([128, 4, D], dt.float16)
            for j in range(4):
                nc.gpsimd.indirect_dma_start(
                    out=GB1[:, j, :],
                    out_offset=None,
                    in_=recv[:],
                    in_offset=bass.IndirectOffsetOnAxis(ap=G1[:, j : j + 1], axis=0),
                    bounds_check=nrow - 1,
                    oob_is_err=False,
                )
                nc.gpsimd.indirect_dma_start(
                    out=GB2[:, j, :],
                    out_offset=None,
                    in_=recv[:],
                    in_offset=bass.IndirectOffsetOnAxis(ap=G2[:, j : j + 1], axis=0),
                    bounds_check=nrow - 1,
                    oob_is_err=False,
                )
            ACC = res.tile([128, 4, D], dt.float32)
            outr = out_d.rearrange("(j p) d -> p j d", p=128)
            engs = [nc.sync, nc.scalar, nc.gpsimd, nc.sync]
            for j in range(4):
                nc.vector.tensor_tensor(
                    ACC[:, j : j + 1, :],
                    GB1[:, j : j + 1, :],
                    GB2[:, j : j + 1, :],
                    au.add,
                )
                engs[j].dma_start(outr[:, j : j + 1, :], ACC[:, j : j + 1, :])

    nc.compile()
    return nc


def _route_host(x, gw):
    """Bit-match reference routing: jax fp32 matmul + top_k + softmax on CPU."""
    import jax
    import jax.numpy as jnp

    cpu = jax.devices("cpu")[0]
    with jax.default_device(cpu):
        gate_logits = jnp.asarray(x) @ jnp.asarray(gw)
        top_vals, top_idx = jax.lax.top_k(gate_logits, 2)
        top_w = jax.nn.softmax(top_vals.astype(jnp.float32), axis=1)
    return np.asarray(top_idx), np.asarray(top_w, np.float32)


def kernel(hidden_states, gate_w, w1, w2):
    global LAST_RESULT
    from concourse.bass_utils import run_bass_kernel_spmd

    x = np.ascontiguousarray(np.asarray(hidden_states, dtype=np.float32)).reshape(T, D)
    gw = np.ascontiguousarray(np.asarray(gate_w, dtype=np.float32))
    w1n = np.asarray(w1, dtype=np.float32)
    w2n = np.asarray(w2, dtype=np.float32)

    top_idx, top_w = _route_host(x, gw)

    # per-expert routed-token lists (sorted by token id = destination-major)
    toks, ranks = [], []
    for e in range(NUM_EXPERTS):
        tok = np.where((top_idx[:, 0] == e) | (top_idx[:, 1] == e))[0]
        toks.append(tok)
        ranks.append(np.where(top_idx[tok, 0] == e, 0, 1))
    max_cap = max(len(t) for t in toks)
    max_capd = max(
        int(np.bincount(t // TPC, minlength=N_CORES).max()) for t in toks
    )
    cap = max(1152, -(-max_cap // 128) * 128)
    capd = max(160, -(-max_capd // 16) * 16)
    nrow = N_CORES * capd
    sc_n = cap // 128

    xgt = np.zeros((N_CORES, 128, 8, cap), np.float16)
    sidx = np.full((N_CORES, 128, sc_n), nrow, np.int32)
    g1 = np.zeros((N_CORES, 128, 4), np.int32)
    g2 = np.zeros((N_CORES, 128, 4), np.int32)

    for e in range(NUM_EXPERTS):
        tok, r = toks[e], ranks[e]
        g = top_w[tok, r]
        xg = (x[tok] * g[:, None]).astype(np.float16)  # [n, D]
        n = len(tok)
        XG = np.zeros((cap, D), np.float16)
        XG[:n] = xg
        xgt[e] = XG.T.reshape(8, 128, cap).transpose(1, 0, 2)
        dest = tok // TPC
        pos = np.zeros(n, np.int64)
        for c in range(N_CORES):
            m = dest == c
            pos[m] = np.arange(m.sum())
        rowidx = dest * capd + pos  # row in send/recv
        sidx[e].reshape(-1)[
            (np.arange(n) % 128) * sc_n + (np.arange(n) // 128)
        ] = rowidx
        lt = tok % TPC
        for c in range(N_CORES):
            m = dest == c
            lr, lw = lt[m], rowidx[m] - c * capd + e * capd
            rr = r[m]
            tgt1, tgt2 = lr[rr == 0], lr[rr == 1]
            g1[c, tgt1 % 128, tgt1 // 128] = lw[rr == 0]
            g2[c, tgt2 % 128, tgt2 // 128] = lw[rr == 1]

    key = (cap, capd)
    if key not in _NC_CACHE:
        _NC_CACHE[key] = _build_nc(cap, capd)
    nc = _NC_CACHE[key]

    in_maps = []
    for e in range(N_CORES):
        w1p = np.ascontiguousarray(
            w1n[e]
            .reshape(8, 128, 8, 4, 128)
            .transpose(2, 1, 3, 0, 4)
            .astype(np.float16)
        )
        w2p = np.ascontiguousarray(
            w2n[e].reshape(8, 4, 128, D).transpose(0, 2, 1, 3).astype(np.float16)
        )
        in_maps.append(
            {
                "xgt": np.ascontiguousarray(xgt[e]),
                "w1p": w1p,
                "w2p": w2p,
                "sidx": np.ascontiguousarray(sidx[e]),
                "g1": np.ascontiguousarray(g1[e]),
                "g2": np.ascontiguousarray(g2[e]),
            }
        )

    trace = bool(os.environ.get("MOE_TRACE"))
    LAST_RESULT = run_bass_kernel_spmd(
        nc, in_maps, core_ids=list(range(N_CORES)), trace=trace
    )

    out = np.empty((T, D), dtype=np.float32)
    for c in range(N_CORES):
        out[c * TPC : (c + 1) * TPC] = LAST_RESULT.results[c]["out"]
    return out.reshape(B, S, D)


# revision 4
# speedup vs baseline: 1.0130x; 1.0130x over previous
"""Expert-parallel sparse top-2 MoE on 8 TRN2 NeuronCores.

One expert per core. Routing/top-2/softmax run on host (bit-matching the
reference's jax-on-CPU ops); each expert's routed tokens are pre-gathered,
pre-gated (g*x, valid since softmax weights are positive and relu is
positively homogeneous), transposed to K-major fp16, and shipped to the
expert's core. The core runs the FFN (mm1 [K=1024]->relu->mm2 [K=4096])
over CAP token slots, indirect-scatters output rows into a per-destination
padded send buffer, and one 8-core AllToAll delivers every token's two
expert contributions to its owner core, which gathers both rows and adds.

Weight DMA per core drops 8x vs token-parallel (only its own expert).
"""

import os

import numpy as np

NUM_EXPERTS = 8
D = 1024
F = 4096
B, S = 2, 2048
T = B * S
N_CORES = 8
TPC = T // N_CORES  # tokens owned per core (output shard)

LAST_RESULT = None
_NC_CACHE = {}


def _build_nc(cap, capd):
    import concourse.mybir as mybir
    import concourse.tile as tile
    from concourse import bacc, bass

    dt = mybir.dt
    au = mybir.AluOpType
    nrow = N_CORES * capd
    sc_n = cap // 128  # slot chunks
    nc = bacc.Bacc("TRN2", target_bir_lowering=False, debug=False, num_devices=N_CORES)

    xgt_d = nc.dram_tensor("xgt", [128, 8, cap], dt.float16, kind="ExternalInput").ap()
    w1_d = nc.dram_tensor("w1p", [8, 128, 4, 8, 128], dt.float16, kind="ExternalInput").ap()
    w2_d = nc.dram_tensor("w2p", [8, 128, 4, D], dt.float16, kind="ExternalInput").ap()
    sidx_d = nc.dram_tensor("sidx", [128, sc_n], dt.int32, kind="ExternalInput").ap()
    g1_d = nc.dram_tensor("g1", [128, 4], dt.int32, kind="ExternalInput").ap()
    g2_d = nc.dram_tensor("g2", [128, 4], dt.int32, kind="ExternalInput").ap()
    out_d = nc.dram_tensor("out", [TPC, D], dt.float32, kind="ExternalOutput").ap()

    # mm1 moving-dim blocks covering cap columns
    nblocks = []
    off = 0
    while off < cap:
        nn = min(512, cap - off)
        nblocks.append((off, nn))
        off += nn
    # mm2 slot-chunk passes (3 chunks x 2 halves = 6 PSUM banks; a 3-chunk
    # pass keeps per-ft matmul work above the W2 stream DMA time)
    passes = [list(range(i, min(i + 3, sc_n))) for i in range(0, sc_n, 3)]

    with tile.TileContext(nc) as tc:
        with (
            tc.tile_pool(name="res", bufs=1) as res,
            tc.tile_pool(name="w1pool", bufs=2) as w1pool,
            tc.tile_pool(name="w2pool", bufs=2) as w2pool,
            tc.tile_pool(name="ogpool", bufs=2) as ogpool,
            tc.tile_pool(name="dram", bufs=1, space="DRAM") as drampool,
        ):
            send = drampool.tile([nrow, D], dt.float16, tag="send")
            recv = drampool.tile([nrow, D], dt.float16, tag="recv")

            # Warm up the collectives path concurrently with compute: the
            # first collective after NEFF load pays ~100us of firmware
            # init/sync; a tiny dummy AllToAll absorbs it under mm1.
            dummy_s = drampool.tile([N_CORES, 16], dt.float32, tag="dummy_s")
            dummy_r = drampool.tile([N_CORES, 16], dt.float32, tag="dummy_r")
            nc.gpsimd.collective_compute(
                "AllToAll",
                au.bypass,
                replica_groups=[list(range(N_CORES))],
                ins=[dummy_s.opt()],
                outs=[dummy_r.opt()],
            )

            # Inputs ride separate engine queues so the first W1 chunk (on
            # sync) isn't queued behind the 2.4 MB XgT load: mm1 can start
            # as soon as w1[fc0] + xgt[ko0] land. Per-ko XgT chunks let the
            # fc0 accumulation begin before the full activation load.
            XgT = res.tile([128, 8, cap], dt.float16)
            for ko in range(8):
                nc.scalar.dma_start(XgT[:, ko, :], xgt_d[:, ko, :])
            SIDX = res.tile([128, sc_n], dt.int32)
            nc.gpsimd.dma_start(SIDX[:], sidx_d[:])
            G1 = res.tile([128, 4], dt.int32)
            nc.gpsimd.dma_start(G1[:], g1_d[:])
            G2 = res.tile([128, 4], dt.int32)
            nc.gpsimd.dma_start(G2[:], g2_d[:])

            Hg = res.tile([128, 32, cap], dt.float16)

            # ---- mm1 + relu: Hg[f, slot] = relu(w1.T @ xg) ----
            with tc.tile_pool(name="psum_h", bufs=2, space="PSUM") as psum_h:
                for fc in range(32):
                    if fc % 4 == 0:
                        W1C4 = w1pool.tile([128, 4, 8, 128], dt.float16, tag="w1c")
                        if fc == 0:
                            # first fc-chunk lands alone so mm1 starts after
                            # 256 KB instead of the full 1 MB group
                            nc.sync.dma_start(W1C4[:, 0:1], w1_d[0, :, 0:1])
                            nc.sync.dma_start(W1C4[:, 1:4], w1_d[0, :, 1:4])
                        else:
                            nc.sync.dma_start(W1C4[:], w1_d[fc // 4])
                    phs = [
                        psum_h.tile(
                            [128, nn], dt.float32, tag=f"ph{nb}", name=f"ph{nb}"
                        )
                        for nb, (_, nn) in enumerate(nblocks)
                    ]
                    for ko in range(8):
                        for nb, (n0, nn) in enumerate(nblocks):
                            nc.tensor.matmul(
                                phs[nb][:],
                                W1C4[:, fc % 4, ko, :],
                                XgT[:, ko, n0 : n0 + nn],
                                start=(ko == 0),
                                stop=(ko == 7),
                            )
                    for nb, (n0, nn) in enumerate(nblocks):
                        nc.scalar.activation(
                            Hg[:, fc, n0 : n0 + nn],
                            phs[nb][:],
                            mybir.ActivationFunctionType.Relu,
                        )

            # ---- mm2: out rows per slot chunk; scatter to send ----
            with tc.tile_pool(name="psum_o", bufs=1, space="PSUM") as psum_o:
                for pi, scs in enumerate(passes):
                    pos = {}
                    for sc in scs:
                        for dh in range(2):
                            # rotate across 4 tag groups (8 banks) so a new
                            # pass accumulates into banks the previous pass
                            # isn't still evacuating
                            tg = (3 * pi + (sc - scs[0])) % 4
                            pos[(sc, dh)] = psum_o.tile(
                                [128, 512],
                                dt.float32,
                                tag=f"po{tg}_{dh}",
                                name=f"po{tg}_{dh}",
                            )
                    for ft in range(32):
                        if ft % 4 == 0:
                            W2T4 = w2pool.tile([128, 4, D], dt.float16, tag="w2t")
                            nc.sync.dma_start(W2T4[:], w2_d[ft // 4])
                        for sc in scs:
                            for dh in range(2):
                                nc.tensor.matmul(
                                    pos[(sc, dh)][:],
                                    Hg[:, ft, sc * 128 : (sc + 1) * 128],
                                    W2T4[:, ft % 4, dh * 512 : (dh + 1) * 512],
                                    start=(ft == 0),
                                    stop=(ft == 31),
                                )
                    for sc in scs:
                        OGC = ogpool.tile([128, D], dt.float16, tag="og")
                        # one half per engine so both evacuate in parallel
                        nc.vector.tensor_copy(OGC[:, 0:512], pos[(sc, 0)][:])
                        nc.scalar.copy(OGC[:, 512:1024], pos[(sc, 1)][:])
                        nc.gpsimd.indirect_dma_start(
                            out=send[:],
                            out_offset=bass.IndirectOffsetOnAxis(
                                ap=SIDX[:, sc : sc + 1], axis=0
                            ),
                            in_=OGC[:],
                            in_offset=None,
                            bounds_check=nrow - 1,
                            oob_is_err=False,
                        )

            # ---- AllToAll: deliver rows to token-owner cores ----
            nc.gpsimd.collective_compute(
                "AllToAll",
                au.bypass,
                replica_groups=[list(range(N_CORES))],
                ins=[send.opt()],
                outs=[recv.opt()],
            )

            # ---- combine: out[t] = recv[g1[t]] + recv[g2[t]] ----
            GB1 = res.tile([128, 4, D], dt.float16)
            GB2 = res.tile([128, 4, D], dt.float16)
            for j in range(4):
                nc.gpsimd.indirect_dma_start(
                    out=GB1[:, j, :],
                    out_offset=None,
                    in_=recv[:],
                    in_offset=bass.IndirectOffsetOnAxis(ap=G1[:, j : j + 1], axis=0),
                    bounds_check=nrow - 1,
                    oob_is_err=False,
                )
                nc.gpsimd.indirect_dma_start(
                    out=GB2[:, j, :],
                    out_offset=None,
                    in_=recv[:],
                    in_offset=bass.IndirectOffsetOnAxis(ap=G2[:, j : j + 1], axis=0),
                    bounds_check=nrow - 1,
                    oob_is_err=False,
                )
            ACC = res.tile([128, 4, D], dt.float32)
            outr = out_d.rearrange("(j p) d -> p j d", p=128)
            engs = [nc.sync, nc.scalar, nc.gpsimd, nc.sync]
            for j in range(4):
                nc.vector.tensor_tensor(
                    ACC[:, j : j + 1, :],
                    GB1[:, j : j + 1, :],
                    GB2[:, j : j + 1, :],
                    au.add,
                )
                engs[j].dma_start(outr[:, j : j + 1, :], ACC[:, j : j + 1, :])

    nc.compile()
    return nc


def _route_host(x, gw):
    """Bit-match reference routing: jax fp32 matmul + top_k + softmax on CPU."""
    import jax
    import jax.numpy as jnp

    cpu = jax.devices("cpu")[0]
    with jax.default_device(cpu):
        gate_logits = jnp.asarray(x) @ jnp.asarray(gw)
        top_vals, top_idx = jax.lax.top_k(gate_logits, 2)
        top_w = jax.nn.softmax(top_vals.astype(jnp.float32), axis=1)
    return np.asarray(top_idx), np.asarray(top_w, np.float32)


def kernel(hidden_states, gate_w, w1, w2):
    global LAST_RESULT
    from concourse.bass_utils import run_bass_kernel_spmd

    x = np.ascontiguousarray(np.asarray(hidden_states, dtype=np.float32)).reshape(T, D)
    gw = np.ascontiguousarray(np.asarray(gate_w, dtype=np.float32))
    w1n = np.asarray(w1, dtype=np.float32)
    w2n = np.asarray(w2, dtype=np.float32)

    top_idx, top_w = _route_host(x, gw)

    # per-expert routed-token lists (sorted by token id = destination-major)
    toks, ranks = [], []
    for e in range(NUM_EXPERTS):
        tok = np.where((top_idx[:, 0] == e) | (top_idx[:, 1] == e))[0]
        toks.append(tok)
        ranks.append(np.where(top_idx[tok, 0] == e, 0, 1))
    max_cap = max(len(t) for t in toks)
    max_capd = max(
        int(np.bincount(t // TPC, minlength=N_CORES).max()) for t in toks
    )
    cap = max(1152, -(-max_cap // 128) * 128)
    capd = max(156, -(-max_capd // 4) * 4)
    nrow = N_CORES * capd
    sc_n = cap // 128

    xgt = np.zeros((N_CORES, 128, 8, cap), np.float16)
    sidx = np.full((N_CORES, 128, sc_n), nrow, np.int32)
    g1 = np.zeros((N_CORES, 128, 4), np.int32)
    g2 = np.zeros((N_CORES, 128, 4), np.int32)

    for e in range(NUM_EXPERTS):
        tok, r = toks[e], ranks[e]
        g = top_w[tok, r]
        xg = (x[tok] * g[:, None]).astype(np.float16)  # [n, D]
        n = len(tok)
        XG = np.zeros((cap, D), np.float16)
        XG[:n] = xg
        xgt[e] = XG.T.reshape(8, 128, cap).transpose(1, 0, 2)
        dest = tok // TPC
        pos = np.zeros(n, np.int64)
        for c in range(N_CORES):
            m = dest == c
            pos[m] = np.arange(m.sum())
        rowidx = dest * capd + pos  # row in send/recv
        sidx[e].reshape(-1)[
            (np.arange(n) % 128) * sc_n + (np.arange(n) // 128)
        ] = rowidx
        lt = tok % TPC
        for c in range(N_CORES):
            m = dest == c
            lr, lw = lt[m], rowidx[m] - c * capd + e * capd
            rr = r[m]
            tgt1, tgt2 = lr[rr == 0], lr[rr == 1]
            g1[c, tgt1 % 128, tgt1 // 128] = lw[rr == 0]
            g2[c, tgt2 % 128, tgt2 // 128] = lw[rr == 1]

    key = (cap, capd)
    if key not in _NC_CACHE:
        _NC_CACHE[key] = _build_nc(cap, capd)
    nc = _NC_CACHE[key]

    in_maps = []
    for e in range(N_CORES):
        w1p = np.ascontiguousarray(
            w1n[e]
            .reshape(8, 128, 8, 4, 128)
            .transpose(2, 1, 3, 0, 4)
            .astype(np.float16)
        )
        w2p = np.ascontiguousarray(
            w2n[e].reshape(8, 4, 128, D).transpose(0, 2, 1, 3).astype(np.float16)
        )
        in_maps.append(
            {
                "xgt": np.ascontiguousarray(xgt[e]),
                "w1p": w1p,
                "w2p": w2p,
                "sidx": np.ascontiguousarray(sidx[e]),
                "g1": np.ascontiguousarray(g1[e]),
                "g2": np.ascontiguousarray(g2[e]),
            }
        )

    trace = bool(os.environ.get("MOE_TRACE"))
    LAST_RESULT = run_bass_kernel_spmd(
        nc, in_maps, core_ids=list(range(N_CORES)), trace=trace
    )

    out = np.empty((T, D), dtype=np.float32)
    for c in range(N_CORES):
        out[c * TPC : (c + 1) * TPC] = LAST_RESULT.results[c]["out"]
    return out.reshape(B, S, D)
